# revision 18
# baseline (speedup 1.0000x reference)
"""Trainium2 Bass kernel: 6+6 layer encoder-decoder classify transformer.

Sharding: pure data-parallel over batch (B=32 -> 4 sequences per core,
8 cores, no collectives). Activations feature-major in SBUF.

v2: encoder QKV/WO/FFN and decoder cross K/V run fp8e4 DoubleRow
matmuls (per-output-column weight scales, descaled at psum eviction via
the Activation engine's per-partition scale AP). LayerNorm emits fp8
directly; stats matmuls read the f32r residual in place. Attention is
bf16 with A/B head matmuls interleaved so the two 64-partition tiles
run concurrently in the PE array; softmax denominators land at psum
partitions 0/64 (column-tiled), and the 1/den broadcast carries the
fp8 output scale. Elementwise work is spread across Act/DVE/GpSimd.
"""

import math
import sys

import numpy as np

for _p in ("/opt/trn_rl_repo",):
    if _p not in sys.path:
        sys.path.append(_p)

import ml_dtypes  # noqa: E402
import concourse.bass as bass  # noqa: E402,F401
import concourse.mybir as mybir  # noqa: E402
import concourse.tile as tile  # noqa: E402
from concourse import bacc  # noqa: E402
from concourse.bass_utils import run_bass_kernel_spmd  # noqa: E402

BF = mybir.dt.bfloat16
F32 = mybir.dt.float32
F32R = mybir.dt.float32r
FP8 = mybir.dt.float8e4
AF = mybir.ActivationFunctionType
ALU = mybir.AluOpType
PM = mybir.MatmulPerfMode

B, S, D, H, FF, NL, V, C = 32, 512, 512, 8, 2048, 6, 32000, 16
EPS = 1e-6
NCORES = 8
BL = B // NCORES          # 4 sequences per core
T = BL * S                # 2048 encoder tokens per core
TD = BL * C               # 64 decoder tokens per core
DK = D // H               # 64
NF = D // 128             # 4 feature tiles
NKP = NF // 2             # 2 DoubleRow k-tile pairs over D
NFF = FF // 128           # 16
NFP = NFF // 2            # 8 DoubleRow pairs over FF
NTK = S // 128            # 4 key-token tiles per sequence
SQD = math.sqrt(D)
ISQDK = 1.0 / math.sqrt(DK)
S_T = 16.0                # fp8 scale for LN outputs
S_H = 32.0                # fp8 scale for relu hidden
S_O = 32.0                # fp8 scale for attention outputs
FP8MAX = 240.0

_CACHE = {}


def _pos_encoding(L):
    pos = np.arange(L, dtype=np.float32)[:, None]
    div = np.exp(np.arange(0, D, 2, dtype=np.float32) * (-math.log(10000.0) / D))
    pe = np.zeros((L, D), np.float32)
    pe[:, 0::2] = np.sin(pos * div)
    pe[:, 1::2] = np.cos(pos * div)
    return pe


# ---------------------------------------------------------------------------
# device kernel builder
# ---------------------------------------------------------------------------

def build_nc(n_enc=NL, n_dec=NL, dbg=()):
    nc = bacc.Bacc("TRN2", target_bir_lowering=False, debug=False,
                   num_devices=NCORES)

    def din(name, shape, dt=BF):
        return nc.dram_tensor(name, list(shape), dt, kind="ExternalInput").ap()

    x0T = din("x0T", (128, NF, T))
    peT = din("peT", (128, NF, S))
    y0T = din("y0T", (128, NF, TD))
    W = {}
    # encoder fp8 weights: [NL, 128, NKP, 2, out]
    for nm in ("wq", "wk", "wv", "wo"):
        W["e" + nm] = din("e" + nm, (NL, 128, NKP, 2, D), FP8)
    W["ew1"] = din("ew1", (NL, 128, NKP, 2, FF), FP8)
    W["ew2"] = din("ew2", (NL, 128, NFP, 2, D), FP8)
    # per-out-column descale vectors (alpha) and biases, feature-tiled
    for nm in ("aq", "ak", "bq", "bk", "ao", "bo", "a2", "b2"):
        W["e" + nm] = din("e" + nm, (NL, 128, NF), F32)
    W["eav"] = din("eav", (NL, 128, 1), F32)
    W["ea1"] = din("ea1", (NL, 128, NFF), F32)
    W["eb1"] = din("eb1", (NL, 128, NFF), F32)
    # decoder bf16 weights (self + cross q/o + ffn)
    for p in ("d", "s"):
        for nm in ("wq", "wk", "wv", "wo"):
            W[p + nm] = din(p + nm, (NL, D, D))
        for nm in ("bq", "bk"):
            W[p + nm] = din(p + nm, (NL, 64, H), F32)
        W[p + "bo"] = din(p + "bo", (NL, 128, NF), F32)
    # decoder cross K/V fp8 over encoder states
    W["swk8"] = din("swk8", (NL, 128, NKP, 2, D), FP8)
    W["swv8"] = din("swv8", (NL, 128, NKP, 2, D), FP8)
    W["sak"] = din("sak", (NL, 64, H), F32)
    W["sbk8"] = din("sbk8", (NL, 64, H), F32)
    W["sav"] = din("sav", (NL, 128, 1), F32)
    W["dw1"] = din("dw1", (NL, D, FF))
    W["db1"] = din("db1", (NL, 128, NFF), F32)
    W["dw2"] = din("dw2", (NL, FF, D))
    W["db2"] = din("db2", (NL, 128, NF), F32)
    genw = din("genw", (128, C * NF, C))
    genb = din("genb", (BL, C), F32)
    out_d = nc.dram_tensor("out", [BL, C], F32, kind="ExternalOutput").ap()
    dbg_d = {}
    for name in dbg:
        shp = {"x": (NF, 128, T), "y": (NF, 128, TD),
               "t": (NF, 128, T)}[name]
        dbg_d[name] = nc.dram_tensor("dbg_" + name, list(shp), F32,
                                     kind="ExternalOutput").ap()

    with tile.TileContext(nc) as tc:
        with tc.tile_pool(name="sb", bufs=1) as sbp, \
             tc.tile_pool(name="pp", bufs=1, space="PSUM") as ppp:
            _body(nc, tc, sbp, ppp, x0T, peT, y0T, W, genw, genb,
                  out_d, dbg_d, n_enc, n_dec)
            import os
            if os.environ.get("KPOOLDBG"):
                print(f"[pools] sb={sbp.current_size() / 128 / 1024:.1f} "
                      f"KB/part  pp={ppp.current_size() / 128 / 1024:.1f}",
                      flush=True)
                for tag, meta in sorted(
                        sbp.tag_meta.items(),
                        key=lambda kv: -kv[1].size_in_bytes() * kv[1].bufs):
                    sz = meta.size_in_bytes() * meta.bufs / 128
                    if sz >= 1024:
                        print(f"  {tag}: {sz / 1024:.1f}KB bufs={meta.bufs}")

    nc.compile()
    return nc


def _body(nc, tc, sbp, ppp, x0T, peT, y0T, W, genw, genb, out_d, dbg_d,
          n_enc, n_dec):
    ctx_lp = nc.allow_low_precision(
        reason="fp8 matmuls + bf16 softmax denominators intentional")
    if hasattr(ctx_lp, "__enter__"):
        ctx_lp.__enter__()
    dma = nc.sync.dma_start

    def st(shape, dt, tag, bufs=1):
        return sbp.tile(shape, dt, tag=tag, bufs=bufs, name=tag)

    def pt(shape, tag, bufs=1):
        return ppp.tile(shape, F32, tag=tag, bufs=bufs, name=tag)

    # constants
    ones_col = st([128, 1], BF, "ones_col")
    nc.vector.memset(ones_col[:], 1.0)
    ones_row = st([1, 128], BF, "ones_row")
    nc.vector.memset(ones_row[:], 1.0)
    negones_row = st([1, 128], BF, "negones_row")
    nc.vector.memset(negones_row[:], -1.0)
    so_row = st([1, 64], BF, "so_row")
    nc.vector.memset(so_row[:], S_O)
    ones_f = st([128, 1], F32, "ones_f")
    nc.vector.memset(ones_f[:], 1.0)
    ones_r = st([128, 1], F32R, "ones_r")
    nc.vector.tensor_copy(ones_r[:], ones_f[:])

    # ---------------- embedding (host-gathered) + positional encoding -----
    peT_sb = st([128, NF, S], BF, "w18")   # parked in the w1 slot
    dma(peT_sb[:], peT[:])

    x = [st([128, T], F32R, f"x{f}") for f in range(NF)]
    for b in range(BL):
        for f in range(NF):
            xg = st([128, S], BF, "rtmp", bufs=2)
            dma(xg[:], x0T[:, f, b * S:(b + 1) * S])
            nc.vector.tensor_add(x[f][:, b * S:(b + 1) * S],
                                 xg[:], peT_sb[:, f, :])

    # ---------------- helpers ---------------------------------------------

    def load_vec(dram, i, ncols, tag):
        v = st([128, ncols], F32, tag, bufs=2)
        dma(v[:], dram[i, :, :])
        return v

    def load_vec_h(dram, i, tag):
        v = st([64, H], F32, tag, bufs=2)
        dma(v[:], dram[i, :, :])
        return v

    def load_w8(dram, i, nkp, nout, tag):
        w = st([128, nkp, 2, nout], FP8, tag, bufs=1)
        dma(w[:], dram[i])
        return w

    def load_wbf(dram, i, nk, nfree, tag):
        ts = []
        for k in range(nk):
            w = st([128, nfree], BF, f"{tag}{k}")
            dma(w[:], dram[i, k * 128:(k + 1) * 128, :])
            ts.append(w)
        return ts

    def ln_fp8(xt, tw, tout):
        """feature-major LN over f32r tiles -> fp8 tile [128, NF, tw]
        with output scale S_T folded in."""
        for c0 in range(0, tw, 512):
            cw = min(512, tw - c0)
            cs = slice(c0, c0 + cw)
            sp = pt([1, 2, cw], "sb")
            for f in range(NF):
                nc.tensor.matmul(sp[:, 0, :], ones_r[:], xt[f][:, cs],
                                 start=(f == 0), stop=(f == NF - 1))
            sqs = []
            for f in range(NF):
                q = st([128, cw], F32R, f"sq{f & 1}", bufs=1)
                eng = nc.gpsimd if f % 2 == 0 else nc.vector
                eng.tensor_mul(q[:], xt[f][:, cs], xt[f][:, cs])
                sqs.append(q)
            for f in range(NF):
                nc.tensor.matmul(sp[:, 1, :], ones_r[:], sqs[f][:],
                                 start=(f == 0), stop=(f == NF - 1))
            m = st([1, cw], F32, "lnm")
            nc.vector.tensor_scalar_mul(m[:], sp[:, 0, :], 1.0 / D)
            v2 = st([1, cw], F32, "lnv")
            nc.vector.tensor_scalar_mul(v2[:], sp[:, 1, :], 1.0 / D)
            msq = st([1, cw], F32, "lnmsq")
            nc.vector.tensor_mul(msq[:], m[:], m[:])
            nc.vector.tensor_sub(v2[:], v2[:], msq[:])
            nc.vector.tensor_scalar_mul(v2[:], v2[:], D / (D - 1.0))
            nc.scalar.sqrt(v2[:], v2[:])
            nc.vector.tensor_scalar_add(v2[:], v2[:], EPS)
            inv = st([1, cw], F32, "lninv")
            nc.vector.reciprocal(inv[:], v2[:])
            invb = st([1, cw], BF, "lninvb")
            nc.vector.tensor_scalar_mul(invb[:], inv[:], S_T)
            nmb = st([1, cw], BF, "lnnmb")
            nc.vector.tensor_mul(nmb[:], m[:], invb[:])
            ab = pt([128, 2, cw], "ab")
            nc.tensor.matmul(ab[:, 0, :], ones_row[:], invb[:])
            nc.tensor.matmul(ab[:, 1, :], negones_row[:], nmb[:])
            for f in range(NF):
                tmp = st([128, cw], BF, "lntmp", bufs=2)
                nc.vector.tensor_mul(tmp[:], xt[f][:, cs], ab[:, 0, :])
                nc.vector.tensor_add(tout[:, f, cs], tmp[:], ab[:, 1, :])

    def layernorm_bf(xt, tw, otag, obufs=1):
        """decoder-side LN: f32 tiles -> bf16 tiles (old scheme, small)."""
        t = [st([128, tw], BF, f"{otag}{f}", bufs=obufs) for f in range(NF)]
        cw = tw
        cs = slice(0, tw)
        sp = pt([1, 2, cw], "sb")
        for f in range(NF):
            nc.tensor.matmul(sp[:, 0, :], ones_r[:], xt[f][:, cs],
                             start=(f == 0), stop=(f == NF - 1))
        sqs = []
        for f in range(NF):
            q = st([128, cw], F32R, f"sq{f & 1}", bufs=1)
            eng = nc.gpsimd if f % 2 == 0 else nc.vector
            eng.tensor_mul(q[:], xt[f][:, cs], xt[f][:, cs])
            sqs.append(q)
        for f in range(NF):
            nc.tensor.matmul(sp[:, 1, :], ones_r[:], sqs[f][:],
                             start=(f == 0), stop=(f == NF - 1))
        m = st([1, cw], F32, "lnm")
        nc.vector.tensor_scalar_mul(m[:], sp[:, 0, :], 1.0 / D)
        v2 = st([1, cw], F32, "lnv")
        nc.vector.tensor_scalar_mul(v2[:], sp[:, 1, :], 1.0 / D)
        msq = st([1, cw], F32, "lnmsq")
        nc.vector.tensor_mul(msq[:], m[:], m[:])
        nc.vector.tensor_sub(v2[:], v2[:], msq[:])
        nc.vector.tensor_scalar_mul(v2[:], v2[:], D / (D - 1.0))
        nc.scalar.sqrt(v2[:], v2[:])
        nc.vector.tensor_scalar_add(v2[:], v2[:], EPS)
        inv = st([1, cw], F32, "lninv")
        nc.vector.reciprocal(inv[:], v2[:])
        invb = st([1, cw], BF, "lninvb")
        nc.vector.tensor_copy(invb[:], inv[:])
        nmb = st([1, cw], BF, "lnnmb")
        nc.vector.tensor_mul(nmb[:], m[:], invb[:])
        ab = pt([128, 2, cw], "ab")
        nc.tensor.matmul(ab[:, 0, :], ones_row[:], invb[:])
        nc.tensor.matmul(ab[:, 1, :], negones_row[:], nmb[:])
        for f in range(NF):
            tmp = st([128, cw], BF, "lntmp", bufs=2)
            nc.vector.tensor_mul(tmp[:], xt[f][:, cs], ab[:, 0, :])
            nc.vector.tensor_add(t[f][:, cs], tmp[:], ab[:, 1, :])
        return t

    # ---------------- encoder ---------------------------------------------

    def proj_fm8(t8, sl, w8, a_ap, b_ap, otag, obufs=1):
        """fp8 DR projection, feature-major out: 4 x [128, 512] bf16."""
        out = [st([128, 512], BF, f"{otag}{m2}", bufs=obufs)
               for m2 in range(NF)]
        for m2 in range(NF):
            ps = pt([128, 512], "ps", bufs=3)
            for kp in range(NKP):
                nc.tensor.matmul(
                    ps[:], w8[:, kp, :, m2 * 128:(m2 + 1) * 128],
                    t8[:, 2 * kp:2 * kp + 2, sl],
                    start=(kp == 0), stop=(kp == NKP - 1),
                    perf_mode=PM.DoubleRow)
            nc.scalar.activation(out[m2][:], ps[:], AF.Identity,
                                 bias=b_ap[:, m2:m2 + 1],
                                 scale=a_ap[:, m2:m2 + 1])
        return out

    def proj_tm8(t8, b, wv8, av):
        """fp8 DR V projection, token-major: NTK x [128, 512] bf16."""
        out = []
        for tt in range(NTK):
            ps = pt([128, 512], "ps", bufs=3)
            t0 = b * S + tt * 128
            for kp in range(NKP):
                nc.tensor.matmul(
                    ps[:], t8[:, 2 * kp:2 * kp + 2, t0:t0 + 128],
                    wv8[:, kp, :, :],
                    start=(kp == 0), stop=(kp == NKP - 1),
                    perf_mode=PM.DoubleRow)
            o = st([128, 512], BF, f"v{tt}", bufs=1)
            nc.vector.tensor_scalar(o[:], ps[:], av[:, 0:1], None,
                                    op0=ALU.mult)
            out.append(o)
        return out

    def attention_b(qb, kb, vb, oT, b):
        """self-attn seq b: A/B head matmuls interleaved (tile concurrency).
        qb/kb feature-major 4 x [128,512]; vb token-major 4 x [128,512];
        writes oT[:, hp, b*S:(b+1)*S] fp8 (scale S_O folded)."""
        for hp in range(H // 2):
            eT = []
            for i in range(NTK):
                psA = pt([128, 512], "ps", bufs=3)
                psB = pt([128, 512], "ps", bufs=3)
                nc.tensor.matmul(
                    psA[:], kb[hp][0:64, i * 128:(i + 1) * 128],
                    qb[hp][0:64, :], start=True, stop=True)
                nc.tensor.matmul(
                    psB[:], kb[hp][64:128, i * 128:(i + 1) * 128],
                    qb[hp][64:128, :], start=True, stop=True)
                e = st([128, 2, 512], BF, f"e{i}", bufs=1)
                nc.scalar.activation(e[:, 0, :], psA[:], AF.Exp, scale=ISQDK)
                nc.scalar.activation(e[:, 1, :], psB[:], AF.Exp, scale=ISQDK)
                eT.append(e)
            s_ps = pt([128, 512], "sb")
            for i in range(NTK):
                nc.tensor.matmul(s_ps[0:1, :], ones_col[:], eT[i][:, 0, :],
                                 start=(i == 0), stop=(i == NTK - 1))
                nc.tensor.matmul(s_ps[64:65, :], ones_col[:], eT[i][:, 1, :],
                                 start=(i == 0), stop=(i == NTK - 1))
            invA = st([1, 512], BF, "ainvA", bufs=2)
            nc.vector.reciprocal(invA[:], s_ps[0:1, :])
            invB = st([1, 512], BF, "ainvB", bufs=2)
            nc.vector.reciprocal(invB[:], s_ps[64:65, :])
            bc = pt([128, 512], "ab")
            nc.tensor.matmul(bc[0:64, :], so_row[:], invA[:])
            nc.tensor.matmul(bc[64:128, :], so_row[:], invB[:])
            bsb = st([128, 512], BF, "bsb", bufs=1)
            nc.scalar.copy(bsb[:], bc[:])
            o_ps = pt([128, 512], "ops", bufs=1)
            for i in range(NTK):
                nc.tensor.matmul(
                    o_ps[0:64, :], vb[i][:, hp * 128:hp * 128 + 64],
                    eT[i][:, 0, :], start=(i == 0), stop=(i == NTK - 1))
                nc.tensor.matmul(
                    o_ps[64:128, :], vb[i][:, hp * 128 + 64:hp * 128 + 128],
                    eT[i][:, 1, :], start=(i == 0), stop=(i == NTK - 1))
            nc.vector.tensor_mul(oT[:, hp, b * S:(b + 1) * S],
                                 o_ps[:], bsb[:])

    def out_proj8(xt, oT8, w8, a_ap, b_ap, tw):
        for m2 in range(NF):
            for c0 in range(0, tw, 512):
                cw = min(512, tw - c0)
                cs = slice(c0, c0 + cw)
                ps = pt([128, cw], "ps", bufs=3)
                for kp in range(NKP):
                    nc.tensor.matmul(
                        ps[:], w8[:, kp, :, m2 * 128:(m2 + 1) * 128],
                        oT8[:, 2 * kp:2 * kp + 2, cs],
                        start=(kp == 0), stop=(kp == NKP - 1),
                        perf_mode=PM.DoubleRow)
                tmp = st([128, cw], BF, "rtmp", bufs=2)
                nc.scalar.activation(tmp[:], ps[:], AF.Identity,
                                     bias=b_ap[:, m2:m2 + 1],
                                     scale=a_ap[:, m2:m2 + 1])
                nc.vector.tensor_add(xt[m2][:, cs], xt[m2][:, cs], tmp[:])

    t8 = st([128, NF, T], FP8, "t8")
    oT8 = st([128, NF, T], FP8, "ot8")

    def enc_layer(i):
        wq = load_w8(W["ewq"], i, NKP, D, "wq8")
        wk = load_w8(W["ewk"], i, NKP, D, "wk8")
        wv = load_w8(W["ewv"], i, NKP, D, "wv8")
        wo = load_w8(W["ewo"], i, NKP, D, "wo8")
        aq = load_vec(W["eaq"], i, NF, "aq")
        bq = load_vec(W["ebq"], i, NF, "bq")
        ak = load_vec(W["eak"], i, NF, "ak")
        bk = load_vec(W["ebk"], i, NF, "bk")
        av = load_vec(W["eav"], i, 1, "av")
        ao = load_vec(W["eao"], i, NF, "ao")
        bo = load_vec(W["ebo"], i, NF, "bo")
        ln_fp8(x, T, t8[:])
        for b in range(BL):
            sl = slice(b * S, (b + 1) * S)
            qb = proj_fm8(t8[:], sl, wq[:], aq[:], bq[:], "qb")
            kb = proj_fm8(t8[:], sl, wk[:], ak[:], bk[:], "kb")
            vb = proj_tm8(t8[:], b, wv[:], av[:])
            attention_b(qb, kb, vb, oT8[:], b)
        out_proj8(x, oT8[:], wo[:], ao[:], bo[:], T)
        # FFN
        w1 = load_w8(W["ew1"], i, NKP, FF, "w18")
        a1 = load_vec(W["ea1"], i, NFF, "a1")
        b1 = load_vec(W["eb1"], i, NFF, "b1")
        w2 = load_w8(W["ew2"], i, NFP, D, "w28")
        a2 = load_vec(W["ea2"], i, NF, "a2")
        b2 = load_vec(W["eb2"], i, NF, "b2")
        ln_fp8(x, T, t8[:])
        for quart in range(4):
            h0 = quart * 512
            hsl = slice(h0, h0 + 512)
            h8 = st([128, NFF, 512], FP8, "h8", bufs=1)
            for ff in range(NFF):
                ps = pt([128, 512], "ps", bufs=3)
                for kp in range(NKP):
                    nc.tensor.matmul(
                        ps[:], w1[:, kp, :, ff * 128:(ff + 1) * 128],
                        t8[:, 2 * kp:2 * kp + 2, hsl],
                        start=(kp == 0), stop=(kp == NKP - 1),
                        perf_mode=PM.DoubleRow)
                nc.scalar.activation(h8[:, ff, :], ps[:], AF.Relu,
                                     bias=b1[:, ff:ff + 1],
                                     scale=a1[:, ff:ff + 1])
            for m2 in range(NF):
                ps = pt([128, 512], "ps", bufs=3)
                for kp in range(NFP):
                    nc.tensor.matmul(
                        ps[:], w2[:, kp, :, m2 * 128:(m2 + 1) * 128],
                        h8[:, 2 * kp:2 * kp + 2, :],
                        start=(kp == 0), stop=(kp == NFP - 1),
                        perf_mode=PM.DoubleRow)
                tmp = st([128, 512], BF, "rtmp", bufs=2)
                nc.scalar.activation(tmp[:], ps[:], AF.Identity,
                                     bias=b2[:, m2:m2 + 1],
                                     scale=a2[:, m2:m2 + 1])
                nc.vector.tensor_add(x[m2][:, hsl], x[m2][:, hsl], tmp[:])

    for i in range(n_enc):
        enc_layer(i)

    if "x" in dbg_d:
        for f in range(NF):
            xc = st([128, T], F32, "xdbg")
            nc.vector.tensor_copy(xc[:], x[f][:])
            dma(dbg_d["x"][f], xc[:])

    # ---------------- decoder ---------------------------------------------
    encl8 = st([128, NF, T], FP8, "encl8")
    ln_fp8(x, T, encl8[:])

    y = [st([128, TD], F32R, f"y{f}") for f in range(NF)]
    y0_sb = st([128, NF, TD], BF, "y0sb")
    dma(y0_sb[:], y0T[:])
    for f in range(NF):
        nc.vector.tensor_copy(y[f][:], y0_sb[:, f, :])

    def proj_hm(src, wt, bt, otag):
        """decoder q/k: head-major [64, H*TD] bf16, head h at cols h*TD."""
        out = st([64, H * TD], BF, otag, bufs=2)
        for h in range(H):
            ps = pt([64, TD], "ps", bufs=3)
            for k in range(NF):
                nc.tensor.matmul(ps[0:64, :], wt[k][:, h * 64:(h + 1) * 64],
                                 src[k][:, 0:TD],
                                 start=(k == 0), stop=(k == NF - 1))
            nc.scalar.activation(out[0:64, h * TD:(h + 1) * TD], ps[0:64, :],
                                 AF.Identity, bias=bt[:, h:h + 1])
        return out

    def proj_hmk8(sl, wk8, a_ap, b_ap, otag):
        """cross keys for one sequence from encl8: head-major [64, H*S]."""
        out = st([64, H * S], BF, otag, bufs=1)
        for h in range(H):
            ps = pt([64, 512], "ps", bufs=3)
            for kp in range(NKP):
                nc.tensor.matmul(
                    ps[0:64, :], wk8[:, kp, :, h * 64:(h + 1) * 64],
                    encl8[:, 2 * kp:2 * kp + 2, sl],
                    start=(kp == 0), stop=(kp == NKP - 1),
                    perf_mode=PM.DoubleRow)
            nc.scalar.activation(out[0:64, h * S:(h + 1) * S],
                                 ps[0:64, :], AF.Identity,
                                 bias=b_ap[:, h:h + 1], scale=a_ap[:, h:h + 1])
        return out

    def proj_tm8_enc(b, wv8, av):
        """cross V from encl8 (fp8 DR): NTK x [128, 512] bf16."""
        out = []
        for tt in range(NTK):
            ps = pt([128, 512], "ps", bufs=3)
            t0 = b * S + tt * 128
            for kp in range(NKP):
                nc.tensor.matmul(
                    ps[:], encl8[:, 2 * kp:2 * kp + 2, t0:t0 + 128],
                    wv8[:, kp, :, :],
                    start=(kp == 0), stop=(kp == NKP - 1),
                    perf_mode=PM.DoubleRow)
            o = st([128, 512], BF, f"v{tt}", bufs=1)
            nc.vector.tensor_scalar(o[:], ps[:], av[:, 0:1], None,
                                    op0=ALU.mult)
            out.append(o)
        return out

    def proj_tm_dec(src, wt):
        """decoder self v: per-sequence token-major tiles [C, D] bf16."""
        out = []
        for b in range(BL):
            ps = pt([C, D], "ps", bufs=3)
            for k in range(NF):
                nc.tensor.matmul(ps[:], src[k][:, b * C:(b + 1) * C], wt[k][:],
                                 start=(k == 0), stop=(k == NF - 1))
            o = st([C, D], BF, f"vd{b}")
            nc.scalar.copy(o[:], ps[:])
            out.append(o)
        return out

    def attention_small(q, kk, v, oT):
        """decoder self-attn: tq=tk=C per sequence, per-b batched heads."""
        for b in range(BL):
            e_ps = pt([C, H * C], "ps", bufs=3)
            for h in range(H):
                nc.tensor.matmul(
                    e_ps[:, h * C:(h + 1) * C],
                    kk[0:64, h * TD + b * C:h * TD + (b + 1) * C],
                    q[0:64, h * TD + b * C:h * TD + (b + 1) * C],
                    start=True, stop=True)
            eS = st([C, H * C], BF, f"ed{b}", bufs=2)
            nc.scalar.activation(eS[:], e_ps[:], AF.Exp, scale=ISQDK)
            s_ps = pt([1, H * C], "sb")
            nc.tensor.matmul(s_ps[:], ones_col[:C, :], eS[:], start=True,
                             stop=True)
            invb = st([1, H * C], BF, "ainvA", bufs=2)
            nc.vector.reciprocal(invb[:], s_ps[:])
            bps = pt([C, H * C], "ab")
            nc.tensor.matmul(bps[:], ones_row[:, :C], invb[:])
            p = st([C, H * C], BF, f"pd{b}", bufs=2)
            nc.vector.tensor_mul(p[:], eS[:], bps[:])
            for hp in range(H // 2):
                o_ps = pt([128, C], "ops", bufs=1)
                for hh in range(2):
                    h = hp * 2 + hh
                    nc.tensor.matmul(
                        o_ps[hh * 64:hh * 64 + 64, :],
                        v[b][:, h * 64:(h + 1) * 64],
                        p[:, h * C:(h + 1) * C],
                        start=True, stop=True)
                nc.scalar.copy(oT[hp][:, b * C:(b + 1) * C], o_ps[:])

    def attention_cross_b(qd, oT, kch, vcb, b):
        """cross attn seq b: tq=C (dec), tk=S (enc), heads batched."""
        eT = []
        s_ps = pt([1, H * C], "sb")
        for i in range(NTK):
            ps = pt([128, H * C], "ps", bufs=3)
            for h in range(H):
                nc.tensor.matmul(
                    ps[:, h * C:(h + 1) * C],
                    kch[0:64, h * S + i * 128:h * S + (i + 1) * 128],
                    qd[0:64, h * TD + b * C:h * TD + (b + 1) * C],
                    start=True, stop=True)
            e = st([128, H * C], BF, f"p{i}", bufs=2)
            nc.scalar.activation(e[:], ps[:], AF.Exp, scale=ISQDK)
            eT.append(e)
            nc.tensor.matmul(s_ps[:], ones_col[:], e[:],
                             start=(i == 0), stop=(i == NTK - 1))
        invb = st([1, H * C], BF, "ainvA", bufs=2)
        nc.vector.reciprocal(invb[:], s_ps[:])
        bps = pt([128, H * C], "ab")
        nc.tensor.matmul(bps[:], ones_row[:], invb[:])
        pb = []
        for i in range(NTK):
            p_ = st([128, H * C], BF, f"p{i}", bufs=2)
            nc.vector.tensor_mul(p_[:], eT[i][:], bps[:])
            pb.append(p_)
        for hp in range(H // 2):
            o_ps = pt([128, C], "ops", bufs=1)
            for hh in range(2):
                h = hp * 2 + hh
                for i in range(NTK):
                    nc.tensor.matmul(
                        o_ps[hh * 64:hh * 64 + 64, :],
                        vcb[i][:, h * 64:(h + 1) * 64],
                        pb[i][:, h * C:(h + 1) * C],
                        start=(i == 0), stop=(i == NTK - 1))
            nc.scalar.copy(oT[hp][:, b * C:(b + 1) * C], o_ps[:])

    def out_proj_residual_bf(yt, tw, oT, wo, bo):
        for m2 in range(NF):
            ps = pt([128, tw], "ps", bufs=3)
            for k in range(NF):
                nc.tensor.matmul(ps[:], wo[k][:, m2 * 128:(m2 + 1) * 128],
                                 oT[k][:, 0:tw], start=(k == 0),
                                 stop=(k == NF - 1))
            nc.vector.scalar_tensor_tensor(
                yt[m2][:, 0:tw], ps[:], bo[:, m2:m2 + 1], yt[m2][:, 0:tw],
                op0=ALU.add, op1=ALU.add)

    def dec_layer(i):
        # ---- self attention
        wq = load_wbf(W["dwq"], i, NF, D, "dwq")
        wk = load_wbf(W["dwk"], i, NF, D, "dwk")
        wv = load_wbf(W["dwv"], i, NF, D, "dwv")
        wo = load_wbf(W["dwo"], i, NF, D, "dwo")
        bq = load_vec_h(W["dbq"], i, "bqh")
        bk = load_vec_h(W["dbk"], i, "bkh")
        bo = load_vec(W["dbo"], i, NF, "bo")
        t = layernorm_bf(y, TD, "td")
        qd = proj_hm(t, wq, bq, "qd")
        kd = proj_hm(t, wk, bk, "kd")
        vd = proj_tm_dec(t, wv)
        oT = [st([128, TD], BF, f"od{f}") for f in range(NF)]
        attention_small(qd, kd, vd, oT)
        out_proj_residual_bf(y, TD, oT, wo, bo)
        # ---- cross attention
        wq = load_wbf(W["swq"], i, NF, D, "dwq")
        wo = load_wbf(W["swo"], i, NF, D, "dwo")
        wk8 = load_w8(W["swk8"], i, NKP, D, "swk8")
        wv8 = load_w8(W["swv8"], i, NKP, D, "swv8")
        bq = load_vec_h(W["sbq"], i, "bqh")
        ak = load_vec_h(W["sak"], i, "akh")
        bk = load_vec_h(W["sbk8"], i, "bkh")
        av = load_vec(W["sav"], i, 1, "av")
        bo = load_vec(W["sbo"], i, NF, "bo")
        t = layernorm_bf(y, TD, "td")
        qd = proj_hm(t, wq, bq, "qd")
        oT = [st([128, TD], BF, f"od{f}") for f in range(NF)]
        for b in range(BL):
            sl = slice(b * S, (b + 1) * S)
            kch = proj_hmk8(sl, wk8[:], ak[:], bk[:], "kch")
            vcb = proj_tm8_enc(b, wv8[:], av[:])
            attention_cross_b(qd, oT, kch, vcb, b)
        out_proj_residual_bf(y, TD, oT, wo, bo)
        # ---- FFN
        w1 = load_wbf(W["dw1"], i, NF, FF, "dw1")
        b1 = load_vec(W["db1"], i, NFF, "b1d")
        b2 = load_vec(W["db2"], i, NF, "b2d")
        t2 = layernorm_bf(y, TD, "td")
        ht = []
        for ff in range(NFF):
            ps = pt([128, TD], "ps", bufs=3)
            for k in range(NF):
                nc.tensor.matmul(ps[:], w1[k][:, ff * 128:(ff + 1) * 128],
                                 t2[k][:], start=(k == 0), stop=(k == NF - 1))
            hh = st([128, TD], BF, f"hd{ff}")
            nc.scalar.activation(hh[:], ps[:], AF.Relu, bias=b1[:, ff:ff + 1])
            ht.append(hh)
        # w2 contraction in 2 half-loads of 8 k-tiles (SBUF pressure);
        # first half parked in SBUF, second half added with bias.
        htmp = []
        for half in range(2):
            w2h = []
            for k in range(8):
                w = st([128, D], BF, f"dw2{k}")
                dma(w[:], W["dw2"][i, (half * 8 + k) * 128:
                                   (half * 8 + k + 1) * 128, :])
                w2h.append(w)
            for m2 in range(NF):
                ps = pt([128, TD], "ps", bufs=3)
                for k in range(8):
                    nc.tensor.matmul(
                        ps[:], w2h[k][:, m2 * 128:(m2 + 1) * 128],
                        ht[half * 8 + k][:],
                        start=(k == 0), stop=(k == 7))
                if half == 0:
                    ht0 = st([128, TD], F32, f"hh0{m2}")
                    nc.scalar.copy(ht0[:], ps[:])
                    htmp.append(ht0)
                else:
                    nc.vector.scalar_tensor_tensor(
                        y[m2][:], ps[:], b2[:, m2:m2 + 1], y[m2][:],
                        op0=ALU.add, op1=ALU.add)
                    nc.vector.tensor_add(y[m2][:], y[m2][:], htmp[m2][:])

    for i in range(n_dec):
        dec_layer(i)

    if "y" in dbg_d:
        for f in range(NF):
            yc = st([128, TD], F32, "ydbg")
            nc.vector.tensor_copy(yc[:], y[f][:])
            dma(dbg_d["y"][f], yc[:])

    # ---------------- generator + log softmax ------------------------------
    dec = layernorm_bf(y, TD, "td")  # bf16 [4][128, 64]
    gw = st([128, C * NF, C], BF, "gw")
    dma(gw[:], genw[:])
    gb = st([BL, C], F32, "gb")
    dma(gb[:], genb[:])
    lg_ps = pt([BL, C], "ps", bufs=3)
    for c in range(C):
        for f in range(NF):
            dslc = dec[f].rearrange("p (b c) -> p c b", c=C)[:, c, :]
            nc.tensor.matmul(lg_ps[:], dslc, gw[:, c * NF + f, :],
                             start=(c == 0 and f == 0),
                             stop=(c == C - 1 and f == NF - 1))
    lg = st([BL, C], F32, "lg")
    nc.vector.tensor_add(lg[:], lg_ps[:], gb[:])
    mx = st([BL, 1], F32, "mx")
    nc.vector.reduce_max(mx[:], lg[:], axis=mybir.AxisListType.X)
    z = st([BL, C], F32, "z")
    nc.vector.tensor_scalar(z[:], lg[:], mx[:], None, op0=ALU.subtract)
    ex = st([BL, C], F32, "ex")
    se = st([BL, 1], F32, "se")
    nc.scalar.activation(ex[:], z[:], AF.Exp, accum_out=se[:])
    ln_s = st([BL, 1], F32, "lns")
    nc.scalar.activation(ln_s[:], se[:], AF.Ln)
    res = st([BL, C], F32, "res")
    nc.vector.tensor_scalar(res[:], z[:], ln_s[:], None, op0=ALU.subtract)
    dma(out_d[:], res[:])


# ---------------------------------------------------------------------------
# host side
# ---------------------------------------------------------------------------

def _q8_cols(w, s_x):
    """Quantize w [K, M] with per-column absmax scales.

    Returns (w8 [K, M] fp8, alpha [M] f32) with w8 = w * s_col and
    alpha = 1 / (s_col * s_x) so that alpha * (w8^T x*s_x) = w^T x.
    """
    amax = np.abs(w).max(axis=0)
    amax = np.maximum(amax, 1e-20)
    s_col = FP8MAX / amax
    w8 = np.clip(w * s_col[None, :], -FP8MAX, FP8MAX).astype(
        ml_dtypes.float8_e4m3)
    return w8, (1.0 / (s_col * s_x)).astype(np.float32)


def _q8_tensor(w, s_x):
    """Per-tensor quantization for moving-operand weights."""
    amax = max(np.abs(w).max(), 1e-20)
    s = FP8MAX / amax
    w8 = np.clip(w * s, -FP8MAX, FP8MAX).astype(ml_dtypes.float8_e4m3)
    return w8, np.float32(1.0 / (s * s_x))


def _pack_dr(w8):
    """[K=512, M] fp8 -> [128, NKP, 2, M] DoubleRow layout."""
    K, M = w8.shape
    nkp = K // 256
    return np.ascontiguousarray(
        w8.reshape(nkp, 2, 128, M).transpose(2, 0, 1, 3))


def _pack_vec(v):  # (dim,) -> (128, dim/128) feature-tiled
    dim = v.shape[0]
    return np.ascontiguousarray(
        v.reshape(dim // 128, 128).T).astype(np.float32)


def _pack_vec_h(v):  # (D,) -> (64, H) head-major
    return np.ascontiguousarray(
        v.reshape(H, 64).T).astype(np.float32)


def prep_host(inputs):
    f = np.asarray

    def bf(a):
        return np.ascontiguousarray(a, dtype=np.float32).astype(
            ml_dtypes.bfloat16)

    common = {}
    pe_s = _pos_encoding(S)          # (S, D)
    common["peT"] = bf(pe_s.T.reshape(NF, 128, S).transpose(1, 0, 2))
    y0 = f(inputs["tgt_emb"]) * SQD + _pos_encoding(C)   # (C, D)
    y0T = y0.T.reshape(NF, 128, C).transpose(1, 0, 2)    # (128, NF, C)
    common["y0T"] = bf(np.tile(y0T, (1, 1, BL)))         # cols b*C+c -> y0[c]

    def pack_bias(b):  # (NL, dim) -> (NL, 128, dim/128)
        return np.stack([_pack_vec(b[i]) for i in range(b.shape[0])])

    def pack_bias_h(b):  # (NL, D) -> (NL, 64, H)
        return np.stack([_pack_vec_h(b[i]) for i in range(b.shape[0])])

    # ---- encoder fp8 weights
    for nm, s_x in (("wq", S_T), ("wk", S_T)):
        w8l, al = [], []
        for i in range(NL):
            w8, a = _q8_cols(f(inputs[f"e_{nm}"][i]), s_x)
            w8l.append(_pack_dr(w8))
            al.append(_pack_vec(a))
        common["e" + nm] = np.stack(w8l)
        common["ea" + nm[1]] = np.stack(al)
    common["ebq"] = pack_bias(f(inputs["e_bq"]))
    common["ebk"] = pack_bias(f(inputs["e_bk"]))
    # v: per-tensor scale (token-major output)
    w8l, al = [], []
    for i in range(NL):
        w8, a = _q8_tensor(f(inputs["e_wv"][i]), S_T)
        w8l.append(_pack_dr(w8))
        al.append(np.full((128, 1), a, np.float32))
    common["ewv"] = np.stack(w8l)
    common["eav"] = np.stack(al)
    # wo: per-col, moving operand is oT (scale S_O); bias folds bv
    w8l, al, bl = [], [], []
    for i in range(NL):
        w8, a = _q8_cols(f(inputs["e_wo"][i]), S_O)
        w8l.append(_pack_dr(w8))
        al.append(_pack_vec(a))
        bo_f = (f(inputs["e_bv"][i]) @ f(inputs["e_wo"][i]) +
                f(inputs["e_bo"][i]))
        bl.append(_pack_vec(bo_f))
    common["ewo"] = np.stack(w8l)
    common["eao"] = np.stack(al)
    common["ebo"] = np.stack(bl)
    # ffn
    w8l, al, bl = [], [], []
    for i in range(NL):
        w8, a = _q8_cols(f(inputs["e_w1"][i]), S_T)
        w8l.append(_pack_dr(w8))
        al.append(_pack_vec(a * S_H))
        bl.append(_pack_vec(f(inputs["e_b1"][i]) * S_H))
    common["ew1"] = np.stack(w8l)
    common["ea1"] = np.stack(al)
    common["eb1"] = np.stack(bl)
    w8l, al = [], []
    for i in range(NL):
        w8, a = _q8_cols(f(inputs["e_w2"][i]), S_H)
        w8l.append(_pack_dr(w8))
        al.append(_pack_vec(a))
    common["ew2"] = np.stack(w8l)
    common["ea2"] = np.stack(al)
    common["eb2"] = pack_bias(f(inputs["e_b2"]))

    # ---- decoder bf16 weights
    for p in ("d", "s"):
        for nm in ("wq", "wk", "wv", "wo"):
            common[p + nm] = bf(f(inputs[f"{p}_{nm}"]))
        common[p + "bq"] = pack_bias_h(f(inputs[f"{p}_bq"]))
        common[p + "bk"] = pack_bias_h(f(inputs[f"{p}_bk"]))
        bo_f = (np.einsum("nd,ndo->no", f(inputs[f"{p}_bv"]),
                          f(inputs[f"{p}_wo"])) + f(inputs[f"{p}_bo"]))
        common[p + "bo"] = pack_bias(bo_f)
    # cross K/V fp8 (over encl8, scale S_T)
    w8l, al = [], []
    for i in range(NL):
        w8, a = _q8_cols(f(inputs["s_wk"][i]), S_T)
        w8l.append(_pack_dr(w8))
        al.append(_pack_vec_h(a))
    common["swk8"] = np.stack(w8l)
    common["sak"] = np.stack(al)
    common["sbk8"] = pack_bias_h(f(inputs["s_bk"]))
    w8l, al = [], []
    for i in range(NL):
        w8, a = _q8_tensor(f(inputs["s_wv"][i]), S_T)
        w8l.append(_pack_dr(w8))
        al.append(np.full((128, 1), a, np.float32))
    common["swv8"] = np.stack(w8l)
    common["sav"] = np.stack(al)
    common["dw1"] = bf(f(inputs["d_w1"]))
    common["db1"] = pack_bias(f(inputs["d_b1"]))
    common["dw2"] = bf(f(inputs["d_w2"]))
    common["db2"] = pack_bias(f(inputs["d_b2"]))
    gw = f(inputs["gen_w"]).reshape(C, NF, 128, C)   # (c, f, p, cls)
    common["genw"] = bf(np.ascontiguousarray(
        gw.transpose(2, 0, 1, 3)).reshape(128, C * NF, C))
    common["genb"] = np.tile(f(inputs["gen_b"])[None, :], (BL, 1)).astype(
        np.float32)

    scr = np.asarray(inputs["scr_x"]).astype(np.int32)
    emb_s = (np.asarray(inputs["src_emb"], dtype=np.float32) * SQD).astype(
        ml_dtypes.bfloat16)
    in_maps = []
    for core in range(NCORES):
        m = dict(common)
        toks = scr[core * BL:(core + 1) * BL].reshape(-1)  # (T,)
        rows = emb_s[toks]                                 # (T, D) bf16
        m["x0T"] = np.ascontiguousarray(
            rows.T.reshape(NF, 128, T).transpose(1, 0, 2))
        in_maps.append(m)
    return in_maps


def kernel(**inputs):
    if "full" not in _CACHE:
        _CACHE["full"] = build_nc()
    nc = _CACHE["full"]
    in_maps = prep_host(inputs)
    res = run_bass_kernel_spmd(nc, in_maps, core_ids=list(range(NCORES)))
    out = np.concatenate([res.results[i]["out"] for i in range(NCORES)],
                         axis=0)
    return out.astype(np.float32)
